# revision 28
# baseline (speedup 1.0000x reference)
"""BPCA pooling layer on 8 Trainium2 NeuronCores (Bass/Tile), bf16 pipeline.

Math: per sample, the reference's `data = patches.reshape(-1, 4)` rows are a
permutation of the sample buffer viewed as [N, 4] (N = H*W*C/4); mean/std/gram
are row-order invariant, so any enumeration of (pixel, channel-group) rows
works for the stats.  The layer is:

  1. per-column mean/std over N rows, dn = (data-mean)/std
  2. gram = dn^T dn (4x4), comp = top eigenvector (jnp.linalg.eigh)
  3. out = (dn @ comp) reshaped to [H/2, W/2, C] with channel permutation

Device plan (2 samples per core, pure data parallel, bf16 inputs -- rel err
~3.4e-3 vs the 2e-2 gate, validated by simulation; both passes are
HBM-bandwidth-bound, ~358 GB/s per core):

  pass 1 (gram): D-layout, k-major groups.  D[r,k] = x.flat[4r+k]; rows
          blocked r = j*128 + p (p = partition).  Groups of 32 j-blocks give
          tiles [128, 130]: cols k*32+jl = D[(g*32+jl)*128+p, k], col 128 =
          ones, col 129 = pad.  Per group ONE bf16 matmul accumulates the
          [128, 130] block-product matrix in PSUM over 196 groups; its
          (k*32+jl, l*32+jl) diagonal entries fold to the 4x4 second-moment
          matrix S, col 128 gives channel sums.  The two samples' chunks are
          interleaved on separate PSUM banks and fed from both HWDGE rings
          (SP + ACT) so the PE never starves and DMA streams continuously.
  host:   fold diag, f64 stats, CPU-jax eigh (same implementation as the
          reference -> same eigenvector sign), w_k = comp_k/std_k,
          bias = -sum mean_k w_k.
  pass 2 (projection): tile-segmented k-plane layout (per segment
          [128, 4*NB], plane k dense).  The whole 4-term MAC runs on the PE:
          psum += (w_k I)^T @ x_k with host-built diag(w_k) stationaries,
          accumulated over k in PSUM; a single DVE tensor_scalar eviction
          adds the bias and converts fp32 -> bf16.  Sample-interleaved fat
          segment loads alternate the SP/ACT rings, stores ride the Pool
          SWDGE ring, so all three DMA paths run concurrently.  bf16 output;
          the host upcasts and unscrambles the layout for free.
"""

import numpy as np

# ---------------------------------------------------------------------------
# Problem constants (hardcoded per spec)
# ---------------------------------------------------------------------------
B, H, W, C = 16, 112, 112, 256
N_CORES = 8
SPC = B // N_CORES          # samples per core = 2
NROWS = H * W * C // 4      # 802816 rows of the [N, 4] data matrix
NBLK = NROWS // 128         # 6272 row-blocks of 128
GRP = 32                    # j-blocks per matmul group
NG = NBLK // GRP            # 196 groups per sample
GC = 4 * GRP + 2            # 130 cols per group: 128 data + ones + pad
HO, WO = H // 2, W // 2     # 56 x 56 output

P1_CHUNKS = [2, 12] + [14] * 13                 # sums to 196; tiny first tile
P2_CHUNKS = [4, 32, 32, 32, 32, 28, 24, 12]     # load segments (groups); sums to 196

_programs = None
LAST_PROFILE = {}
TRACE = False
TRACE_DIRS = {}


def _bf16():
    import ml_dtypes
    return ml_dtypes.bfloat16


# ---------------------------------------------------------------------------
# TileContext helpers
# ---------------------------------------------------------------------------
def _make_tile_context(nc):
    from concourse.tile import TileContext
    return TileContext(nc)


def _split_sync_waits(nc):
    """walrus (CoreV2/V3 codegen) rejects instructions carrying more than 2
    sync commands (waits + updates combined); Tile freely emits e.g. 2 waits
    + 1 update.  Hoist excess waits onto same-engine NOPs inserted directly
    before the offending instruction -- same engine means the same program-
    order point, so semantics are unchanged."""
    import concourse.mybir as mybir

    def mint_nop(engine):
        inner = nc.engines[engine].nop().ins
        for blk in nc.m.functions[0].blocks:
            il = blk.instructions
            for k in range(len(il) - 1, -1, -1):
                if il[k] is inner:
                    il.pop(k)
                    return inner
        raise RuntimeError("minted nop not found in any block")

    for fn in nc.m.functions:
        for blk in fn.blocks:
            il = blk.instructions
            i = 0
            while i < len(il):
                inst = il[i]
                si = inst.sync_info
                waits = list(si.on_wait) if si and si.on_wait else []
                upds = list(si.on_update) if si and si.on_update else []
                # observed walrus limits: at most 1 wait per instruction
                if len(waits) > 1:
                    extra, keep = waits[:-1], waits[-1:]
                    for wchunk in extra:
                        nop = mint_nop(inst.engine)
                        nop.sync_info = mybir.SyncInfo(
                            on_wait=[wchunk], on_update=[])
                        il.insert(i, nop)
                        i += 1
                    inst.sync_info = mybir.SyncInfo(
                        on_wait=keep, on_update=upds)
                i += 1


def _build_pass1():
    import concourse.bass as bass
    import concourse.mybir as mybir

    f32 = mybir.dt.float32
    bf16 = mybir.dt.bfloat16

    nc = bass.Bass("TRN2", target_bir_lowering=False, debug=False,
                   num_devices=N_CORES)
    x = nc.dram_tensor("x", [SPC, 128, NG * GC], bf16,
                       kind="ExternalInput").ap()
    stats = nc.dram_tensor("stats", [SPC, 128, GC], f32,
                           kind="ExternalOutput").ap()

    with _make_tile_context(nc) as tc:
        with (
            tc.tile_pool(name="inp", bufs=6) as inp,
            tc.tile_pool(name="psum", bufs=2, space="PSUM") as psum,
            tc.tile_pool(name="sout", bufs=2) as soutp,
        ):
            ps0 = psum.tile([128, GC], f32, tag="ps0")
            ps1 = psum.tile([128, GC], f32, tag="ps1")
            pss = [ps0, ps1]
            # interleave the two samples' chunks: PE fills one sample's
            # DMA-wait gaps with the other's matmuls (separate PSUM banks),
            # staying busy enough to hold the 2.4 GHz p-state
            g0 = 0
            for ng in P1_CHUNKS:
                for s in range(SPC):
                    t = inp.tile([128, ng * GC], bf16, tag=f"in{s}",
                                 name=f"in{s}")
                    t3 = t[:].rearrange("p (g c) -> p g c", c=GC)
                    # two HWDGE rings feed descriptors concurrently
                    eng = nc.sync if s == 0 else nc.scalar
                    eng.dma_start(
                        out=t[:], in_=x[s, :, g0 * GC:(g0 + ng) * GC])
                    for j in range(ng):
                        nc.tensor.matmul(
                            pss[s][:, 0:GC],
                            t3[:, j, 0:128],
                            t3[:, j, 0:GC],
                            start=(g0 + j == 0),
                            stop=(g0 + j == NG - 1),
                            skip_group_check=True)
                g0 += ng
            for s in range(SPC):
                so = soutp.tile([128, GC], f32, tag=f"so{s}", name=f"so{s}")
                nc.vector.tensor_copy(out=so[:], in_=pss[s][:, 0:GC])
                nc.gpsimd.dma_start(out=stats[s], in_=so[:])
    _split_sync_waits(nc)
    return nc


def _build_pass2():
    import concourse.bass as bass
    import concourse.mybir as mybir

    f32 = mybir.dt.float32
    bf16 = mybir.dt.bfloat16

    nc = bass.Bass("TRN2", target_bir_lowering=False, debug=False,
                   num_devices=N_CORES)
    # tile-segmented k-plane layout: per tile [128, 4*NB] with plane k dense
    x = nc.dram_tensor("x", [SPC, 128, 4 * NBLK], bf16,
                       kind="ExternalInput").ap()
    # stationaries: per (sample, k) a 128x128 diag(w_k) matrix
    wi = nc.dram_tensor("wi", [128, SPC * 4 * 128], bf16,
                        kind="ExternalInput").ap()
    wb = nc.dram_tensor("wb", [128, 16], f32, kind="ExternalInput").ap()
    out = nc.dram_tensor("out", [SPC, 128, NBLK], bf16,
                         kind="ExternalOutput").ap()

    with _make_tile_context(nc) as tc:
        with (
            tc.tile_pool(name="w", bufs=1) as wpool,
            tc.tile_pool(name="inp", bufs=2) as inp,
            tc.tile_pool(name="psum", bufs=4, space="PSUM") as psum,
            tc.tile_pool(name="ev", bufs=2) as evp,
        ):
            wt = wpool.tile([128, 16], f32, tag="wb")
            nc.scalar.dma_start(out=wt[:], in_=wb[:])
            wit = wpool.tile([128, SPC * 4 * 128], bf16, tag="wi")
            nc.scalar.dma_start(out=wit[:], in_=wi[:])
            b0s = [0] * SPC
            # interleave the two samples' segments: continuous DMA stream
            # with no sample-boundary dip
            for ng in P2_CHUNKS:
                for s in range(SPC):
                    bias = wt[:, 8 * s + 4:8 * s + 5]
                    b0 = b0s[s]
                    NBs = ng * GRP
                    # fat segment load: up to 16 KB/partition contiguous
                    t = inp.tile([128, 4 * NBs], bf16, tag=f"it{s}",
                                 name=f"it{s}")
                    # alternate load rings per sample (SP / ACT)
                    (nc.sync if s == 0 else nc.scalar).dma_start(
                        out=t[:], in_=x[s, :, 4 * b0:4 * (b0 + NBs)])
                    evt = evp.tile([128, NBs], bf16, tag=f"ev{s}",
                                   name=f"ev{s}")
                    for j0 in range(0, ng, 16):
                        sub = min(16, ng - j0)
                        NB = sub * GRP
                        # projection on PE: psum += (w_k I)^T @ x_k
                        ps = psum.tile([128, NB], f32, tag="ps")
                        for k in range(4):
                            c0 = (s * 4 + k) * 128
                            o0 = k * NBs + j0 * GRP
                            nc.tensor.matmul(
                                ps[:, 0:NB],
                                wit[:, c0:c0 + 128],
                                t[:, o0:o0 + NB],
                                start=(k == 0), stop=(k == 3),
                                skip_group_check=True)
                        # eviction adds bias, converts fp32 -> bf16 (DVE)
                        nc.vector.tensor_scalar_add(
                            evt[:, j0 * GRP:j0 * GRP + NB], ps[:, 0:NB], bias)
                    # stores on the Pool SWDGE ring (loads own SP/ACT)
                    nc.gpsimd.dma_start(
                        out=out[s, :, b0:b0 + NBs], in_=evt[:])
                    b0s[s] = b0 + NBs
    _split_sync_waits(nc)
    return nc


def _get_programs():
    global _programs
    if _programs is None:
        _programs = (_build_pass1(), _build_pass2())
    return _programs


def _host_prep(x):
    """x [B,H,W,C] f32 -> bf16 D-layout groups [B, 128, NG*GC]."""
    bf16 = _bf16()
    xg = np.empty((B, 128, NG, GC), bf16)
    d = x.reshape(B, NBLK, 128, 4).transpose(0, 2, 1, 3)      # [B,128,j,k]
    d = d.reshape(B, 128, NG, GRP, 4).transpose(0, 1, 2, 4, 3)  # [B,128,g,k,jl]
    xg[..., :128] = d.reshape(B, 128, NG, 128).astype(bf16)
    xg[..., 128] = 1.0
    xg[..., 129] = 0.0
    return xg.reshape(B, 128, NG * GC)


def _host_prep2(x):
    """x [B,H,W,C] f32 -> bf16 tile-segmented k-plane layout [B,128,4*NBLK].

    Per tile (chunk of NB = ng*32 blocks): [128, 4, NB] with plane k dense.
    """
    bf16 = _bf16()
    xp = x.reshape(B, NBLK, 128, 4).transpose(0, 2, 3, 1).astype(bf16)
    xt = np.empty((B, 128, 4 * NBLK), bf16)                # [B,128,k,b]
    b0 = 0
    for ng in P2_CHUNKS:
        NB = ng * GRP
        xt[:, :, 4 * b0:4 * (b0 + NB)] = \
            xp[:, :, :, b0:b0 + NB].reshape(B, 128, 4 * NB)
        b0 += NB
    return xt


def _host_middle(stats):
    """stats: [B, 128, GC] f32 -> w [B, 4] f64, bias [B] f64.

    PSUM[(k*32+jl), (l*32+jl')] = block products; diagonal jl==jl' entries
    fold to S_kl, col 128 folds to channel sums.  Downstream matches the
    reference exactly: gram from (S - N mu mu^T)/(sigma sigma^T), comp =
    eigh(gram f32) top eigenvector on CPU jax.
    """
    st = stats.astype(np.float64)
    S = np.einsum("bkjlj->bkl", st[:, :, :128].reshape(B, 4, GRP, 4, GRP))
    colsum = st[:, :, 128].reshape(B, 4, GRP).sum(axis=2)

    mu = colsum / NROWS
    e2 = np.einsum("bkk->bk", S) / NROWS
    var = np.maximum(e2 - mu * mu, 0.0)
    sigma = np.sqrt(var)
    denom = sigma[:, :, None] * sigma[:, None, :]
    gram = (S - NROWS * mu[:, :, None] * mu[:, None, :])
    with np.errstate(divide="ignore", invalid="ignore"):
        gram = np.where(denom > 0, gram / np.where(denom > 0, denom, 1.0), 0.0)

    # eigh with the same implementation/backend the reference uses (CPU jax)
    import jax
    import jax.numpy as jnp
    with jax.default_device(jax.devices("cpu")[0]):
        V = np.asarray(jnp.linalg.eigh(jnp.asarray(gram, jnp.float32))[1])
    comp = V[:, :, -1].astype(np.float64)                # top eigenvector

    with np.errstate(divide="ignore", invalid="ignore"):
        w = np.where(sigma > 0, comp / np.where(sigma > 0, sigma, 1.0), 0.0)
    bias = -(mu * w).sum(axis=1)
    return w, bias


def kernel(x):
    from concourse.bass_utils import run_bass_kernel_spmd

    x = np.ascontiguousarray(np.asarray(x), dtype=np.float32)
    assert x.shape == (B, H, W, C), x.shape
    nc1, nc2 = _get_programs()
    core_ids = list(range(N_CORES))

    xg = _host_prep(x)
    in1 = [{"x": xg[c * SPC:(c + 1) * SPC]} for c in range(N_CORES)]
    kw1 = dict(trace=True, tmpdir=TRACE_DIRS.get("pass1")) if TRACE else {}
    r1 = run_bass_kernel_spmd(nc1, in1, core_ids, **kw1)
    if TRACE:
        LAST_PROFILE["pass1_ns"] = r1.exec_time_ns
    stats = np.concatenate([r1.results[c]["stats"] for c in range(N_CORES)])

    w, bias = _host_middle(stats)
    wbs = []
    for c in range(N_CORES):
        a = np.zeros((128, 16), np.float32)
        for s in range(SPC):
            b = c * SPC + s
            a[:, 8 * s:8 * s + 4] = w[b].astype(np.float32)
            a[:, 8 * s + 4] = np.float32(bias[b])
        wbs.append(a)

    xt2 = _host_prep2(x)
    bf = _bf16()
    wis = []
    for c in range(N_CORES):
        a = np.zeros((128, SPC, 4, 128), bf)
        idx = np.arange(128)
        for s in range(SPC):
            b = c * SPC + s
            for k in range(4):
                a[idx, s, k, idx] = bf(w[b][k])
        wis.append(a.reshape(128, SPC * 4 * 128))
    in2 = [{"x": xt2[c * SPC:(c + 1) * SPC], "wb": wbs[c], "wi": wis[c]}
           for c in range(N_CORES)]
    kw2 = dict(trace=True, tmpdir=TRACE_DIRS.get("pass2")) if TRACE else {}
    r2 = run_bass_kernel_spmd(nc2, in2, core_ids, **kw2)
    if TRACE:
        LAST_PROFILE["pass2_ns"] = r2.exec_time_ns

    # gather + unscramble: dev out [B, p=(dj,cg), j=(h,wo)] -> [B,HO,WO,C]
    dev = np.concatenate([np.asarray(r2.results[c]["out"])
                          for c in range(N_CORES)])
    dev = dev.astype(np.float32).reshape(B, 2, 64, HO, 2, WO)
    #                 [b, dj, cg, ho, di, wo] -> [b, ho, wo, di, dj, cg]
    out = dev.transpose(0, 3, 5, 4, 1, 2).reshape(B, HO, WO, C)
    return np.ascontiguousarray(out)
